# revision 9
# baseline (speedup 1.0000x reference)
"""YOLO loss kernel for Trainium2 (Bass/Tile), data-parallel over 8 NeuronCores.

Math (per sample n, cell s; S=14, SS=196, B=2, C=20, D=30):
  t4 = target conf channel (binary 0/1, channels 4 and 9 identical)
  All box coords scaled by S (iou is invariant): corner = x +- (S/2)w,
  areas = S^2 * w * h.
  For the box PAIR (channels {o..o+3} for o in {0,5}, strided APs):
    prb = x + (S/2)w ; nlt = (S/2)w - x   (= -lt)
    ox = min(tprb, prb) + min(tnlt, nlt) ; oy likewise
    inter = relu(ox)*max(oy,0) ; union = S^2*pw*ph + S^2*tw*th - inter
    iou = inter * recip(union)
  sel = iou1 > iou0 ; selm = sel*t4 ; s0m = t4 - selm  (mask pair msk=[s0m,selm])
  conf  = sum msk_h*(p_{4+5h} - iou_h)^2          (pair op, channels {4,9})
  noobj = sum 0.5*(1-t4)*(p4^2 + p9^2)            (pair op)
  coord = 5 * sum_k msk_h*(p_{5h+k}-t_{5h+k})^2   (8-channel pair op, bcast msk)
  class = sum t4 * (p_c-t_c)^2, c=10..29          (4-channel chunks, bcast t4)
Masked squares use (mask*e)^2 = mask*e^2 (masks binary); weights fold into the
ACT Square scale. Every reduction is an ACT accum_out into a [128, NSLOT]
partial tile; host sums across slots/partitions/cores and divides by N.

Layout per core: 512 samples = 4 blocks x 128 partitions, processed in passes
of GRPS=[1,2,1] blocks (small first pass -> compute starts early; small last
pass + split last class chunk -> short drain tail). Engine split per the
TimelineSim cost model: DVE all stt/min/cmp + chain ops, Pool (gpsimd) the
independent big sub/mults, ACT all square-accumulate reductions.
"""

import numpy as np

import concourse.mybir as mybir
from concourse import bacc
from concourse.bass_utils import run_bass_kernel_spmd
from concourse.tile import TileContext

F32 = mybir.dt.float32
OP = mybir.AluOpType
AF = mybir.ActivationFunctionType

N, D, S = 4096, 30, 14
SS = S * S          # 196
NCORE = 8
NPC = N // NCORE    # 512 samples per core
P = 128
NBLK = NPC // P     # 4 blocks of 128 samples
GRPS = [1, 2, 1]    # blocks per pass
HALF = S / 2.0      # corner scale
AREA = float(S * S)
SQ5 = 5.0 ** 0.5
SQH = 0.5 ** 0.5

# class chunks (channel counts); last pass splits the tail chunk
CLS_CHUNKS = [[4, 4, 4, 4, 4], [4, 4, 4, 4, 4], [4, 4, 4, 4, 2, 1, 1]]
SLOTS_PER_PASS = [3 + len(c) for c in CLS_CHUNKS]   # noobj, conf, coord + class
NSLOT = sum(SLOTS_PER_PASS)                          # 25

_CACHE = {}


def _build():
    nc = bacc.Bacc("TRN2", target_bir_lowering=False, debug=False)
    pred = nc.dram_tensor("pred", [NPC, D * SS], F32, kind="ExternalInput")
    tgt = nc.dram_tensor("target", [NPC, D * SS], F32, kind="ExternalInput")
    out = nc.dram_tensor("out", [P, NSLOT], F32, kind="ExternalOutput")

    # [NPC, D*SS] -> [P, block, D*SS]; sample = block*128 + p
    pred_r = pred[:, :].rearrange("(a p) d -> p a d", a=NBLK)
    tgt_r = tgt[:, :].rearrange("(a p) d -> p a d", a=NBLK)

    npass = len(GRPS)
    offs = [sum(GRPS[:q]) for q in range(npass)]

    with TileContext(nc) as tc:
        with (
            tc.tile_pool(name="box", bufs=1) as boxp,     # pb/tb per pass
            tc.tile_pool(name="cls", bufs=3) as clsp,     # class chunk streams
            tc.tile_pool(name="tmp", bufs=1) as tmp,      # per-pass temps
            tc.tile_pool(name="accp", bufs=1) as accp,
        ):
            acc = accp.tile([P, NSLOT], F32)

            pb, tb = [], []
            for q, g in enumerate(GRPS):
                a0 = offs[q]
                pbq = boxp.tile([P, g, 10, SS], F32, tag=f"pb{q}", name=f"pb{q}")
                tbq = boxp.tile([P, g, 10, SS], F32, tag=f"tb{q}", name=f"tb{q}")
                # tb padded to 10-channel stride so {0..3,5..8} is a regular AP
                nc.sync.dma_start(
                    out=tbq[:, :, 0:9, :], in_=tgt_r[:, a0:a0 + g, 0:9 * SS])
                nc.sync.dma_start(
                    out=pbq, in_=pred_r[:, a0:a0 + g, 0:10 * SS])
                pb.append(pbq)
                tb.append(tbq)

            # class chunk DMAs (SP queue, streamed after box data)
            pc, tcl = [], []
            for q, g in enumerate(GRPS):
                a0 = offs[q]
                pcq, tcq = [], []
                lo = 10
                for j, w in enumerate(CLS_CHUNKS[q]):
                    pbufs = (4 if g == 1 else 3) if w == 4 else 2
                    tbufs = 5 if w == 4 else 2
                    pj = clsp.tile([P, g, w, SS], F32, tag=f"pc{g}_{w}",
                                   name=f"pc{q}_{j}", bufs=pbufs)
                    tj = clsp.tile([P, g, w, SS], F32, tag=f"tc{g}_{w}",
                                   name=f"tc{q}_{j}", bufs=tbufs)
                    nc.sync.dma_start(
                        out=pj, in_=pred_r[:, a0:a0 + g, lo * SS:(lo + w) * SS])
                    nc.sync.dma_start(
                        out=tj, in_=tgt_r[:, a0:a0 + g, lo * SS:(lo + w) * SS])
                    pcq.append(pj)
                    tcq.append(tj)
                    lo += w
                pc.append(pcq)
                tcl.append(tcq)

            # ---- per-pass state ----
            st = [dict() for _ in range(npass)]

            def slot(q, i):
                base = sum(SLOTS_PER_PASS[:q])
                return acc[:, base + i:base + i + 1]

            def T(q, name, shape):
                t = tmp.tile(shape, F32, tag=f"{name}g{GRPS[q]}",
                             name=f"{name}{q}")
                st[q][name] = t
                return t

            def box_phase(nc, q):
                g = GRPS[q]
                s = st[q]
                pbv = pb[q][:, :, :, :]
                tbv = tb[q][:, :, :, :]
                t4 = tbv[:, :, 4, :]

                # target prep (DVE stt): scaled corners + area
                tpr = T(q, "tpr", [P, g, 2, SS])    # [x-axis, y-axis]
                tnl = T(q, "tnl", [P, g, 2, SS])
                nc.vector.scalar_tensor_tensor(
                    tpr, tbv[:, :, 2:4, :], HALF, tbv[:, :, 0:2, :],
                    OP.mult, OP.add)
                nc.vector.scalar_tensor_tensor(
                    tnl, tbv[:, :, 2:4, :], HALF, tbv[:, :, 0:2, :],
                    OP.mult, OP.subtract)
                tarea = T(q, "tarea", [P, g, SS])
                nc.vector.scalar_tensor_tensor(
                    tarea, tbv[:, :, 2, :], AREA, tbv[:, :, 3, :],
                    OP.mult, OP.mult)

                # w = 1 - t4 (ACT); noobj pair masked by w
                w = T(q, "w", [P, g, SS])
                nc.scalar.activation(w, t4, AF.Copy, bias=1.0, scale=-1.0)
                conf_p = pbv.rearrange("p g (h c) s -> p g h c s", h=2)[:, :, :, 4, :]
                nm = T(q, "ce", [P, g, 2, SS])
                wb2 = w[:, :, :].unsqueeze(2).broadcast_to([P, g, 2, SS])
                nc.gpsimd.tensor_tensor(nm, conf_p, wb2, OP.mult)
                nc.scalar.activation(nm, nm, AF.Square, scale=SQH,
                                     accum_out=slot(q, 0))

                # pred corners, both boxes at once: channel pairs {c, c+5}
                pv = pbv.rearrange("p g (h c) s -> p g h c s", h=2)
                pw_x = pv[:, :, :, 2, :]   # [P, g, 2(box), SS]
                pw_y = pv[:, :, :, 3, :]
                px = pv[:, :, :, 0, :]
                py = pv[:, :, :, 1, :]
                prx = T(q, "prx", [P, g, 2, SS])
                nlx = T(q, "nlx", [P, g, 2, SS])
                pry = T(q, "pry", [P, g, 2, SS])
                nly = T(q, "nly", [P, g, 2, SS])
                nc.vector.scalar_tensor_tensor(prx, pw_x, HALF, px, OP.mult, OP.add)
                nc.vector.scalar_tensor_tensor(nlx, pw_x, HALF, px, OP.mult, OP.subtract)
                nc.vector.scalar_tensor_tensor(pry, pw_y, HALF, py, OP.mult, OP.add)
                nc.vector.scalar_tensor_tensor(nly, pw_y, HALF, py, OP.mult, OP.subtract)
                # pq = pred area (Pool): (S^2 folded in s1 stt via AREA scale)
                pq = T(q, "pq", [P, g, 2, SS])
                nc.gpsimd.tensor_tensor(pq, pw_x, pw_y, OP.mult)

                # intersect: mins (target side broadcast over box pair)
                tprxb = tpr[:, :, 0, :].unsqueeze(2).broadcast_to([P, g, 2, SS])
                tpryb = tpr[:, :, 1, :].unsqueeze(2).broadcast_to([P, g, 2, SS])
                tnlxb = tnl[:, :, 0, :].unsqueeze(2).broadcast_to([P, g, 2, SS])
                tnlyb = tnl[:, :, 1, :].unsqueeze(2).broadcast_to([P, g, 2, SS])
                nc.vector.tensor_tensor(prx, tprxb, prx, OP.min)
                nc.vector.tensor_tensor(nlx, tnlxb, nlx, OP.min)
                nc.vector.tensor_tensor(pry, tpryb, pry, OP.min)
                nc.vector.tensor_tensor(nly, tnlyb, nly, OP.min)
                nc.vector.tensor_add(prx, prx, nlx)          # ox
                nc.vector.tensor_add(pry, pry, nly)          # oy
                nc.scalar.activation(nlx, prx, AF.Relu)      # relu(ox)
                nc.vector.scalar_tensor_tensor(
                    pry, pry, 0.0, nlx, OP.max, OP.mult)     # inter
                # union = (S^2*pq - inter) + tarea
                tareab = tarea[:, :, :].unsqueeze(2).broadcast_to([P, g, 2, SS])
                nc.vector.scalar_tensor_tensor(
                    nly, pq, AREA, pry, OP.mult, OP.subtract)
                nc.vector.tensor_tensor(nly, nly, tareab, OP.add)    # union
                nc.vector.reciprocal_approx_fast(
                    out=prx[:, :, :, :].rearrange("p g h s -> p (g h s)"),
                    in_=nly[:, :, :, :].rearrange("p g h s -> p (g h s)"))
                iou = pq
                st[q]["iou"] = iou
                flex = nc.gpsimd if q == 0 else nc.vector
                flex.tensor_mul(iou, pry, prx)

                # masks
                sel = tarea
                nc.vector.tensor_tensor(
                    sel, iou[:, :, 1, :], iou[:, :, 0, :], OP.is_gt)
                msk = s["tpr"]
                st[q]["msk"] = msk
                flex.tensor_mul(msk[:, :, 1, :], sel, t4)            # selm
                flex.tensor_sub(msk[:, :, 0, :], t4, msk[:, :, 1, :])  # s0m

                # conf pair: (p_conf - iou) * msk
                ce = s["ce"]
                flex.tensor_sub(ce, conf_p, iou)
                flex.tensor_mul(ce, ce, msk)
                nc.scalar.activation(ce, ce, AF.Square, scale=1.0,
                                     accum_out=slot(q, 1))

            def coord_phase(nc, q, sub_eng, mul_eng):
                g = GRPS[q]
                s = st[q]
                pv8 = pb[q][:, :, :, :].rearrange(
                    "p g (h c) s -> p g h c s", h=2)[:, :, :, 0:4, :]
                tv8 = tb[q][:, :, :, :].rearrange(
                    "p g (h c) s -> p g h c s", h=2)[:, :, :, 0:4, :]
                e8 = T(q, "e8", [P, g, 2, 4, SS])
                for h in range(2):
                    sub_eng.tensor_tensor(
                        e8[:, :, h, :, :], pv8[:, :, h, :, :],
                        tv8[:, :, h, :, :], OP.subtract)
                mskb = s["msk"][:, :, :, :].unsqueeze(3).broadcast_to(
                    [P, g, 2, 4, SS])
                mul_eng.tensor_tensor(e8, e8, mskb, OP.mult)
                nc.scalar.activation(e8, e8, AF.Square, scale=SQ5,
                                     accum_out=slot(q, 2))

            def class_chunk(nc, q, j, sub_eng, mul_eng):
                g = GRPS[q]
                w = CLS_CHUNKS[q][j]
                t4 = tb[q][:, :, :, :][:, :, 4, :]
                e = tcl[q][j]
                sub_eng.tensor_tensor(e, pc[q][j], e, OP.subtract)
                t4b = t4.unsqueeze(2).broadcast_to([P, g, w, SS])
                mul_eng.tensor_tensor(e, e, t4b, OP.mult)
                nc.scalar.activation(e, e, AF.Square, scale=1.0,
                                     accum_out=slot(q, 3 + j))

            V, G = nc.vector, nc.gpsimd

            # ---- emission order (per-engine queues are in-order),
            # ---- interleaved by expected readiness ----
            box_phase(nc, 0)
            box_phase(nc, 1)
            coord_phase(nc, 0, G, V)
            box_phase(nc, 2)
            class_chunk(nc, 0, 0, G, V)
            class_chunk(nc, 0, 1, G, V)
            coord_phase(nc, 1, G, V)
            class_chunk(nc, 0, 2, G, V)
            class_chunk(nc, 0, 3, G, V)
            coord_phase(nc, 2, G, V)
            class_chunk(nc, 0, 4, G, V)
            for j in range(len(CLS_CHUNKS[1])):
                sub_eng, mul_eng = (G, G) if j in (0, 2) else (G, V)
                class_chunk(nc, 1, j, sub_eng, mul_eng)
            for j in range(len(CLS_CHUNKS[2])):
                # short tail: final two chunks on disjoint engine chains
                eng = [(G, V), (G, V), (G, V), (G, V), (V, V), (V, V),
                       (G, G)][j]
                class_chunk(nc, 2, j, *eng)

            nc.scalar.dma_start(out=out[:, :], in_=acc)
    nc.compile()
    return nc


def _get_nc():
    if "nc" not in _CACHE:
        _CACHE["nc"] = _build()
    return _CACHE["nc"]


def kernel(pred: np.ndarray, target: np.ndarray) -> np.ndarray:
    nc = _get_nc()
    in_maps = []
    for k in range(NCORE):
        sl = slice(k * NPC, (k + 1) * NPC)
        in_maps.append({
            "pred": np.ascontiguousarray(pred[sl]).reshape(NPC, D * SS),
            "target": np.ascontiguousarray(target[sl]).reshape(NPC, D * SS),
        })
    res = run_bass_kernel_spmd(nc, in_maps, core_ids=list(range(NCORE)))
    total = sum(float(r["out"].astype(np.float64).sum()) for r in res.results)
    return np.float32(total / N)


# revision 10
# speedup vs baseline: 1.0815x; 1.0815x over previous
"""YOLO loss kernel for Trainium2 (Bass/Tile), data-parallel over 8 NeuronCores.

Math (per sample n, cell s; S=14, SS=196, B=2, C=20, D=30):
  t4 = target conf channel (binary 0/1, channels 4 and 9 identical)
  All box coords scaled by S (iou is invariant): corner = x +- (S/2)w,
  areas = S^2 * w * h.
  For the box PAIR (channels {o..o+3} for o in {0,5}, strided APs):
    prb = x + (S/2)w ; nlt = (S/2)w - x   (= -lt)
    ox = min(tprb, prb) + min(tnlt, nlt) ; oy likewise
    inter = relu(ox)*max(oy,0) ; union = S^2*pw*ph + S^2*tw*th - inter
    iou = inter * recip(union)
  sel = iou1 > iou0 ; selm = sel*t4 ; s0m = t4 - selm  (mask pair msk=[s0m,selm])
  conf  = sum msk_h*(p_{4+5h} - iou_h)^2          (pair op, channels {4,9})
  noobj = sum 0.5*(1-t4)*(p4^2 + p9^2)            (pair op)
  coord = 5 * sum_k msk_h*(p_{5h+k}-t_{5h+k})^2   (8-channel pair op, bcast msk)
  class = sum t4 * (p_c-t_c)^2, c=10..29          (4-channel chunks, bcast t4)
Masked squares use (mask*e)^2 = mask*e^2 (masks binary); weights fold into the
ACT Square scale. Every reduction is an ACT accum_out into a [128, NSLOT]
partial tile; host sums across slots/partitions/cores and divides by N.

Layout per core: 512 samples = 4 blocks x 128 partitions, processed in passes
of GRPS=[1,2,1] blocks (small first pass -> compute starts early; small last
pass + split last class chunk -> short drain tail). Engine split per the
TimelineSim cost model: DVE all stt/min/cmp + chain ops, Pool (gpsimd) the
independent big sub/mults, ACT all square-accumulate reductions.
"""

import numpy as np

import concourse.mybir as mybir
from concourse import bacc
from concourse.bass_utils import run_bass_kernel_spmd
from concourse.tile import TileContext

F32 = mybir.dt.float32
OP = mybir.AluOpType
AF = mybir.ActivationFunctionType

N, D, S = 4096, 30, 14
SS = S * S          # 196
NCORE = 8
NPC = N // NCORE    # 512 samples per core
P = 128
NBLK = NPC // P     # 4 blocks of 128 samples
GRPS = [1, 2, 1]    # blocks per pass
HALF = S / 2.0      # corner scale
AREA = float(S * S)
SQ5 = 5.0 ** 0.5
SQH = 0.5 ** 0.5

# class chunks (channel counts); last pass splits the tail chunk
CLS_CHUNKS = [[4, 4, 4, 4, 4], [4, 4, 4, 4, 4], [4, 4, 4, 4, 2, 1, 1]]
SLOTS_PER_PASS = [3 + len(c) for c in CLS_CHUNKS]   # noobj, conf, coord + class
NSLOT = sum(SLOTS_PER_PASS)                          # 25

_CACHE = {}


def _build():
    nc = bacc.Bacc("TRN2", target_bir_lowering=False, debug=False)
    pred = nc.dram_tensor("pred", [NPC, D * SS], F32, kind="ExternalInput")
    tgt = nc.dram_tensor("target", [NPC, D * SS], F32, kind="ExternalInput")
    out = nc.dram_tensor("out", [P, NSLOT], F32, kind="ExternalOutput")

    # [NPC, D*SS] -> [P, block, D*SS]; sample = block*128 + p
    pred_r = pred[:, :].rearrange("(a p) d -> p a d", a=NBLK)
    tgt_r = tgt[:, :].rearrange("(a p) d -> p a d", a=NBLK)

    npass = len(GRPS)
    offs = [sum(GRPS[:q]) for q in range(npass)]

    with TileContext(nc) as tc:
        with (
            tc.tile_pool(name="box", bufs=1) as boxp,     # pb/tb per pass
            tc.tile_pool(name="cls", bufs=3) as clsp,     # class chunk streams
            tc.tile_pool(name="tmp", bufs=1) as tmp,      # per-pass temps
            tc.tile_pool(name="accp", bufs=1) as accp,
        ):
            acc = accp.tile([P, NSLOT], F32)

            pb, tb = [], []
            for q, g in enumerate(GRPS):
                a0 = offs[q]
                pbq = boxp.tile([P, g, 10, SS], F32, tag=f"pb{q}", name=f"pb{q}")
                tbq = boxp.tile([P, g, 10, SS], F32, tag=f"tb{q}", name=f"tb{q}")
                # tb padded to 10-channel stride so {0..3,5..8} is a regular AP
                nc.sync.dma_start(
                    out=tbq[:, :, 0:9, :], in_=tgt_r[:, a0:a0 + g, 0:9 * SS])
                nc.sync.dma_start(
                    out=pbq, in_=pred_r[:, a0:a0 + g, 0:10 * SS])
                pb.append(pbq)
                tb.append(tbq)

            # class chunk DMAs (SP queue, streamed after box data)
            pc, tcl = [], []
            for q, g in enumerate(GRPS):
                a0 = offs[q]
                pcq, tcq = [], []
                lo = 10
                for j, w in enumerate(CLS_CHUNKS[q]):
                    pbufs = (4 if g == 1 else 3) if w == 4 else 2
                    tbufs = 5 if w == 4 else 2
                    pj = clsp.tile([P, g, w, SS], F32, tag=f"pc{g}_{w}",
                                   name=f"pc{q}_{j}", bufs=pbufs)
                    tj = clsp.tile([P, g, w, SS], F32, tag=f"tc{g}_{w}",
                                   name=f"tc{q}_{j}", bufs=tbufs)
                    nc.sync.dma_start(
                        out=pj, in_=pred_r[:, a0:a0 + g, lo * SS:(lo + w) * SS])
                    nc.sync.dma_start(
                        out=tj, in_=tgt_r[:, a0:a0 + g, lo * SS:(lo + w) * SS])
                    pcq.append(pj)
                    tcq.append(tj)
                    lo += w
                pc.append(pcq)
                tcl.append(tcq)

            # ---- per-pass state ----
            st = [dict() for _ in range(npass)]

            def slot(q, i):
                base = sum(SLOTS_PER_PASS[:q])
                return acc[:, base + i:base + i + 1]

            def T(q, name, shape):
                t = tmp.tile(shape, F32, tag=f"{name}g{GRPS[q]}",
                             name=f"{name}{q}")
                st[q][name] = t
                return t

            def box_phase(nc, q):
                g = GRPS[q]
                s = st[q]
                pbv = pb[q][:, :, :, :]
                tbv = tb[q][:, :, :, :]
                t4 = tbv[:, :, 4, :]

                # target prep (DVE stt): scaled corners + area
                tpr = T(q, "tpr", [P, g, 2, SS])    # [x-axis, y-axis]
                tnl = T(q, "tnl", [P, g, 2, SS])
                nc.vector.scalar_tensor_tensor(
                    tpr, tbv[:, :, 2:4, :], HALF, tbv[:, :, 0:2, :],
                    OP.mult, OP.add)
                nc.vector.scalar_tensor_tensor(
                    tnl, tbv[:, :, 2:4, :], HALF, tbv[:, :, 0:2, :],
                    OP.mult, OP.subtract)
                tarea = T(q, "tarea", [P, g, SS])
                nc.vector.scalar_tensor_tensor(
                    tarea, tbv[:, :, 2, :], AREA, tbv[:, :, 3, :],
                    OP.mult, OP.mult)

                # w = 1 - t4 (ACT); noobj pair masked by w
                w = T(q, "w", [P, g, SS])
                nc.scalar.activation(w, t4, AF.Copy, bias=1.0, scale=-1.0)
                conf_p = pbv.rearrange("p g (h c) s -> p g h c s", h=2)[:, :, :, 4, :]
                nm = T(q, "ce", [P, g, 2, SS])
                wb2 = w[:, :, :].unsqueeze(2).broadcast_to([P, g, 2, SS])
                nc.gpsimd.tensor_tensor(nm, conf_p, wb2, OP.mult)
                nc.scalar.activation(nm, nm, AF.Square, scale=SQH,
                                     accum_out=slot(q, 0))

                # pred corners, both boxes at once: channel pairs {c, c+5}
                pv = pbv.rearrange("p g (h c) s -> p g h c s", h=2)
                pw_x = pv[:, :, :, 2, :]   # [P, g, 2(box), SS]
                pw_y = pv[:, :, :, 3, :]
                px = pv[:, :, :, 0, :]
                py = pv[:, :, :, 1, :]
                prx = T(q, "prx", [P, g, 2, SS])
                nlx = T(q, "nlx", [P, g, 2, SS])
                pry = T(q, "pry", [P, g, 2, SS])
                nly = T(q, "nly", [P, g, 2, SS])
                nc.vector.scalar_tensor_tensor(prx, pw_x, HALF, px, OP.mult, OP.add)
                nc.vector.scalar_tensor_tensor(nlx, pw_x, HALF, px, OP.mult, OP.subtract)
                nc.vector.scalar_tensor_tensor(pry, pw_y, HALF, py, OP.mult, OP.add)
                nc.vector.scalar_tensor_tensor(nly, pw_y, HALF, py, OP.mult, OP.subtract)
                # pq = pred area (Pool): (S^2 folded in s1 stt via AREA scale)
                pq = T(q, "pq", [P, g, 2, SS])
                nc.gpsimd.tensor_tensor(pq, pw_x, pw_y, OP.mult)

                # intersect: mins (target side broadcast over box pair)
                tprxb = tpr[:, :, 0, :].unsqueeze(2).broadcast_to([P, g, 2, SS])
                tpryb = tpr[:, :, 1, :].unsqueeze(2).broadcast_to([P, g, 2, SS])
                tnlxb = tnl[:, :, 0, :].unsqueeze(2).broadcast_to([P, g, 2, SS])
                tnlyb = tnl[:, :, 1, :].unsqueeze(2).broadcast_to([P, g, 2, SS])
                nc.vector.tensor_tensor(prx, tprxb, prx, OP.min)
                nc.vector.tensor_tensor(nlx, tnlxb, nlx, OP.min)
                nc.vector.tensor_tensor(pry, tpryb, pry, OP.min)
                nc.vector.tensor_tensor(nly, tnlyb, nly, OP.min)
                nc.vector.tensor_add(prx, prx, nlx)          # ox
                nc.vector.tensor_add(pry, pry, nly)          # oy
                nc.scalar.activation(nlx, prx, AF.Relu)      # relu(ox)
                nc.vector.scalar_tensor_tensor(
                    pry, pry, 0.0, nlx, OP.max, OP.mult)     # inter
                # union = (S^2*pq - inter) + tarea
                tareab = tarea[:, :, :].unsqueeze(2).broadcast_to([P, g, 2, SS])
                nc.vector.scalar_tensor_tensor(
                    nly, pq, AREA, pry, OP.mult, OP.subtract)
                nc.vector.tensor_tensor(nly, nly, tareab, OP.add)    # union
                nc.vector.reciprocal_approx_fast(
                    out=prx[:, :, :, :].rearrange("p g h s -> p (g h s)"),
                    in_=nly[:, :, :, :].rearrange("p g h s -> p (g h s)"))
                iou = pq
                st[q]["iou"] = iou
                nc.vector.tensor_mul(iou, pry, prx)

                # masks
                sel = tarea
                nc.vector.tensor_tensor(
                    sel, iou[:, :, 1, :], iou[:, :, 0, :], OP.is_gt)
                msk = s["tpr"]
                st[q]["msk"] = msk
                nc.vector.tensor_mul(msk[:, :, 1, :], sel, t4)       # selm
                nc.vector.tensor_sub(msk[:, :, 0, :], t4, msk[:, :, 1, :])  # s0m

                # conf pair: (p_conf - iou) * msk
                ce = s["ce"]
                nc.vector.tensor_sub(ce, conf_p, iou)
                nc.vector.tensor_mul(ce, ce, msk)
                nc.scalar.activation(ce, ce, AF.Square, scale=1.0,
                                     accum_out=slot(q, 1))

            def coord_phase(nc, q, sub_eng, mul_eng):
                g = GRPS[q]
                s = st[q]
                pv8 = pb[q][:, :, :, :].rearrange(
                    "p g (h c) s -> p g h c s", h=2)[:, :, :, 0:4, :]
                tv8 = tb[q][:, :, :, :].rearrange(
                    "p g (h c) s -> p g h c s", h=2)[:, :, :, 0:4, :]
                e8 = T(q, "e8", [P, g, 2, 4, SS])
                for h in range(2):
                    sub_eng.tensor_tensor(
                        e8[:, :, h, :, :], pv8[:, :, h, :, :],
                        tv8[:, :, h, :, :], OP.subtract)
                mskb = s["msk"][:, :, :, :].unsqueeze(3).broadcast_to(
                    [P, g, 2, 4, SS])
                mul_eng.tensor_tensor(e8, e8, mskb, OP.mult)
                nc.scalar.activation(e8, e8, AF.Square, scale=SQ5,
                                     accum_out=slot(q, 2))

            def class_chunk(nc, q, j, sub_eng, mul_eng):
                g = GRPS[q]
                w = CLS_CHUNKS[q][j]
                t4 = tb[q][:, :, :, :][:, :, 4, :]
                e = tcl[q][j]
                sub_eng.tensor_tensor(e, pc[q][j], e, OP.subtract)
                t4b = t4.unsqueeze(2).broadcast_to([P, g, w, SS])
                mul_eng.tensor_tensor(e, e, t4b, OP.mult)
                nc.scalar.activation(e, e, AF.Square, scale=1.0,
                                     accum_out=slot(q, 3 + j))

            V, G = nc.vector, nc.gpsimd

            # ---- emission order (per-engine queues are in-order),
            # ---- interleaved by expected readiness ----
            box_phase(nc, 0)
            box_phase(nc, 1)
            coord_phase(nc, 0, G, V)
            box_phase(nc, 2)
            class_chunk(nc, 0, 0, G, V)
            class_chunk(nc, 0, 1, G, V)
            coord_phase(nc, 1, G, V)
            class_chunk(nc, 0, 2, G, V)
            class_chunk(nc, 0, 3, G, V)
            coord_phase(nc, 2, G, V)
            class_chunk(nc, 0, 4, G, V)
            for j in range(len(CLS_CHUNKS[1])):
                sub_eng, mul_eng = (G, G) if j == 2 else (G, V)
                class_chunk(nc, 1, j, sub_eng, mul_eng)
            for j in range(len(CLS_CHUNKS[2])):
                # short tail: final two chunks on disjoint engine chains
                eng = [(G, V), (G, V), (G, V), (G, V), (V, V), (V, V),
                       (G, G)][j]
                class_chunk(nc, 2, j, *eng)

            nc.scalar.dma_start(out=out[:, :], in_=acc)
    nc.compile()
    return nc


def _get_nc():
    if "nc" not in _CACHE:
        _CACHE["nc"] = _build()
    return _CACHE["nc"]


def kernel(pred: np.ndarray, target: np.ndarray) -> np.ndarray:
    nc = _get_nc()
    in_maps = []
    for k in range(NCORE):
        sl = slice(k * NPC, (k + 1) * NPC)
        in_maps.append({
            "pred": np.ascontiguousarray(pred[sl]).reshape(NPC, D * SS),
            "target": np.ascontiguousarray(target[sl]).reshape(NPC, D * SS),
        })
    res = run_bass_kernel_spmd(nc, in_maps, core_ids=list(range(NCORE)))
    total = sum(float(r["out"].astype(np.float64).sum()) for r in res.results)
    return np.float32(total / N)


# revision 11
# speedup vs baseline: 1.0828x; 1.0012x over previous
"""YOLO loss kernel for Trainium2 (Bass/Tile), data-parallel over 8 NeuronCores.

Math (per sample n, cell s; S=14, SS=196, B=2, C=20, D=30):
  t4 = target conf channel (binary 0/1, channels 4 and 9 identical)
  All box coords scaled by S (iou is invariant): corner = x +- (S/2)w,
  areas = S^2 * w * h.
  For the box PAIR (channels {o..o+3} for o in {0,5}, strided APs):
    prb = x + (S/2)w ; nlt = (S/2)w - x   (= -lt)
    ox = min(tprb, prb) + min(tnlt, nlt) ; oy likewise
    inter = relu(ox)*max(oy,0) ; union = S^2*pw*ph + S^2*tw*th - inter
    iou = inter * recip(union)
  sel = iou1 > iou0 ; selm = sel*t4 ; s0m = t4 - selm  (mask pair msk=[s0m,selm])
  conf  = sum msk_h*(p_{4+5h} - iou_h)^2          (pair op, channels {4,9})
  noobj = sum 0.5*(1-t4)*(p4^2 + p9^2)            (pair op)
  coord = 5 * sum_k msk_h*(p_{5h+k}-t_{5h+k})^2   (8-channel pair op, bcast msk)
  class = sum t4 * (p_c-t_c)^2, c=10..29          (4-channel chunks, bcast t4)
Masked squares use (mask*e)^2 = mask*e^2 (masks binary); weights fold into the
ACT Square scale. Every reduction is an ACT accum_out into a [128, NSLOT]
partial tile; host sums across slots/partitions/cores and divides by N.

Layout per core: 512 samples = 4 blocks x 128 partitions, processed in passes
of GRPS=[1,2,1] blocks (small first pass -> compute starts early; small last
pass + split last class chunk -> short drain tail). Engine split per the
TimelineSim cost model: DVE all stt/min/cmp + chain ops, Pool (gpsimd) the
independent big sub/mults, ACT all square-accumulate reductions.
"""

import numpy as np

import concourse.mybir as mybir
from concourse import bacc
from concourse.bass_utils import run_bass_kernel_spmd
from concourse.tile import TileContext

F32 = mybir.dt.float32
OP = mybir.AluOpType
AF = mybir.ActivationFunctionType

N, D, S = 4096, 30, 14
SS = S * S          # 196
NCORE = 8
NPC = N // NCORE    # 512 samples per core
P = 128
NBLK = NPC // P     # 4 blocks of 128 samples
GRPS = [1, 2, 1]    # blocks per pass
HALF = S / 2.0      # corner scale
AREA = float(S * S)
SQ5 = 5.0 ** 0.5
SQH = 0.5 ** 0.5

# class chunks (channel counts); last pass splits the tail chunk
CLS_CHUNKS = [[4, 4, 4, 4, 4], [4, 4, 4, 4, 4], [4, 4, 4, 4, 2, 1, 1]]
SLOTS_PER_PASS = [3 + len(c) for c in CLS_CHUNKS]   # noobj, conf, coord + class
NSLOT = sum(SLOTS_PER_PASS)                          # 25

_CACHE = {}


def _build():
    nc = bacc.Bacc("TRN2", target_bir_lowering=False, debug=False)
    pred = nc.dram_tensor("pred", [NPC, D * SS], F32, kind="ExternalInput")
    tgt = nc.dram_tensor("target", [NPC, D * SS], F32, kind="ExternalInput")
    out = nc.dram_tensor("out", [P, NSLOT], F32, kind="ExternalOutput")

    # [NPC, D*SS] -> [P, block, D*SS]; sample = block*128 + p
    pred_r = pred[:, :].rearrange("(a p) d -> p a d", a=NBLK)
    tgt_r = tgt[:, :].rearrange("(a p) d -> p a d", a=NBLK)

    npass = len(GRPS)
    offs = [sum(GRPS[:q]) for q in range(npass)]

    with TileContext(nc) as tc:
        with (
            tc.tile_pool(name="box", bufs=1) as boxp,     # pb/tb per pass
            tc.tile_pool(name="cls", bufs=3) as clsp,     # class chunk streams
            tc.tile_pool(name="tmp", bufs=1) as tmp,      # per-pass temps
            tc.tile_pool(name="accp", bufs=1) as accp,
        ):
            acc = accp.tile([P, NSLOT], F32)

            pb, tb = [], []
            for q, g in enumerate(GRPS):
                a0 = offs[q]
                pbq = boxp.tile([P, g, 10, SS], F32, tag=f"pb{q}", name=f"pb{q}")
                tbq = boxp.tile([P, g, 10, SS], F32, tag=f"tb{q}", name=f"tb{q}")
                # tb padded to 10-channel stride so {0..3,5..8} is a regular AP
                nc.sync.dma_start(
                    out=tbq[:, :, 0:9, :], in_=tgt_r[:, a0:a0 + g, 0:9 * SS])
                nc.sync.dma_start(
                    out=pbq, in_=pred_r[:, a0:a0 + g, 0:10 * SS])
                pb.append(pbq)
                tb.append(tbq)

            # class chunk DMAs (SP queue, streamed after box data)
            pc, tcl = [], []
            for q, g in enumerate(GRPS):
                a0 = offs[q]
                pcq, tcq = [], []
                lo = 10
                for j, w in enumerate(CLS_CHUNKS[q]):
                    pbufs = (4 if g == 1 else 3) if w == 4 else 2
                    tbufs = 5 if w == 4 else 2
                    pj = clsp.tile([P, g, w, SS], F32, tag=f"pc{g}_{w}",
                                   name=f"pc{q}_{j}", bufs=pbufs)
                    tj = clsp.tile([P, g, w, SS], F32, tag=f"tc{g}_{w}",
                                   name=f"tc{q}_{j}", bufs=tbufs)
                    nc.sync.dma_start(
                        out=pj, in_=pred_r[:, a0:a0 + g, lo * SS:(lo + w) * SS])
                    nc.sync.dma_start(
                        out=tj, in_=tgt_r[:, a0:a0 + g, lo * SS:(lo + w) * SS])
                    pcq.append(pj)
                    tcq.append(tj)
                    lo += w
                pc.append(pcq)
                tcl.append(tcq)

            # ---- per-pass state ----
            st = [dict() for _ in range(npass)]

            def slot(q, i):
                base = sum(SLOTS_PER_PASS[:q])
                return acc[:, base + i:base + i + 1]

            def T(q, name, shape):
                t = tmp.tile(shape, F32, tag=f"{name}g{GRPS[q]}",
                             name=f"{name}{q}")
                st[q][name] = t
                return t

            def box_phase(nc, q):
                g = GRPS[q]
                s = st[q]
                pbv = pb[q][:, :, :, :]
                tbv = tb[q][:, :, :, :]
                t4 = tbv[:, :, 4, :]

                # target prep (DVE stt): scaled corners + area
                tpr = T(q, "tpr", [P, g, 2, SS])    # [x-axis, y-axis]
                tnl = T(q, "tnl", [P, g, 2, SS])
                nc.vector.scalar_tensor_tensor(
                    tpr, tbv[:, :, 2:4, :], HALF, tbv[:, :, 0:2, :],
                    OP.mult, OP.add)
                nc.vector.scalar_tensor_tensor(
                    tnl, tbv[:, :, 2:4, :], HALF, tbv[:, :, 0:2, :],
                    OP.mult, OP.subtract)
                tarea = T(q, "tarea", [P, g, SS])
                nc.vector.scalar_tensor_tensor(
                    tarea, tbv[:, :, 2, :], AREA, tbv[:, :, 3, :],
                    OP.mult, OP.mult)

                # w = 1 - t4 (ACT); noobj pair masked by w
                w = T(q, "w", [P, g, SS])
                nc.scalar.activation(w, t4, AF.Copy, bias=1.0, scale=-1.0)
                conf_p = pbv.rearrange("p g (h c) s -> p g h c s", h=2)[:, :, :, 4, :]
                nm = T(q, "ce", [P, g, 2, SS])
                wb2 = w[:, :, :].unsqueeze(2).broadcast_to([P, g, 2, SS])
                nc.gpsimd.tensor_tensor(nm, conf_p, wb2, OP.mult)
                nc.scalar.activation(nm, nm, AF.Square, scale=SQH,
                                     accum_out=slot(q, 0))

                # pred corners, both boxes at once: channel pairs {c, c+5}
                pv = pbv.rearrange("p g (h c) s -> p g h c s", h=2)
                pw_x = pv[:, :, :, 2, :]   # [P, g, 2(box), SS]
                pw_y = pv[:, :, :, 3, :]
                px = pv[:, :, :, 0, :]
                py = pv[:, :, :, 1, :]
                prx = T(q, "prx", [P, g, 2, SS])
                nlx = T(q, "nlx", [P, g, 2, SS])
                pry = T(q, "pry", [P, g, 2, SS])
                nly = T(q, "nly", [P, g, 2, SS])
                nc.vector.scalar_tensor_tensor(prx, pw_x, HALF, px, OP.mult, OP.add)
                nc.vector.scalar_tensor_tensor(nlx, pw_x, HALF, px, OP.mult, OP.subtract)
                nc.vector.scalar_tensor_tensor(pry, pw_y, HALF, py, OP.mult, OP.add)
                nc.vector.scalar_tensor_tensor(nly, pw_y, HALF, py, OP.mult, OP.subtract)
                # pq = pred area (Pool): (S^2 folded in s1 stt via AREA scale)
                pq = T(q, "pq", [P, g, 2, SS])
                nc.gpsimd.tensor_tensor(pq, pw_x, pw_y, OP.mult)

                # intersect: mins (target side broadcast over box pair)
                tprxb = tpr[:, :, 0, :].unsqueeze(2).broadcast_to([P, g, 2, SS])
                tpryb = tpr[:, :, 1, :].unsqueeze(2).broadcast_to([P, g, 2, SS])
                tnlxb = tnl[:, :, 0, :].unsqueeze(2).broadcast_to([P, g, 2, SS])
                tnlyb = tnl[:, :, 1, :].unsqueeze(2).broadcast_to([P, g, 2, SS])
                nc.vector.tensor_tensor(prx, tprxb, prx, OP.min)
                nc.vector.tensor_tensor(nlx, tnlxb, nlx, OP.min)
                nc.vector.tensor_tensor(pry, tpryb, pry, OP.min)
                nc.vector.tensor_tensor(nly, tnlyb, nly, OP.min)
                nc.vector.tensor_add(prx, prx, nlx)          # ox
                nc.vector.tensor_add(pry, pry, nly)          # oy
                nc.scalar.activation(nlx, prx, AF.Relu)      # relu(ox)
                nc.vector.scalar_tensor_tensor(
                    pry, pry, 0.0, nlx, OP.max, OP.mult)     # inter
                # union = (S^2*pq - inter) + tarea
                tareab = tarea[:, :, :].unsqueeze(2).broadcast_to([P, g, 2, SS])
                nc.vector.scalar_tensor_tensor(
                    nly, pq, AREA, pry, OP.mult, OP.subtract)
                nc.vector.tensor_tensor(nly, nly, tareab, OP.add)    # union
                nc.vector.reciprocal_approx_fast(
                    out=prx[:, :, :, :].rearrange("p g h s -> p (g h s)"),
                    in_=nly[:, :, :, :].rearrange("p g h s -> p (g h s)"))
                iou = pq
                st[q]["iou"] = iou
                nc.vector.tensor_mul(iou, pry, prx)

                # masks
                sel = tarea
                nc.vector.tensor_tensor(
                    sel, iou[:, :, 1, :], iou[:, :, 0, :], OP.is_gt)
                msk = s["tpr"]
                st[q]["msk"] = msk
                nc.vector.tensor_mul(msk[:, :, 1, :], sel, t4)       # selm
                nc.vector.tensor_sub(msk[:, :, 0, :], t4, msk[:, :, 1, :])  # s0m

                # conf pair: (p_conf - iou) * msk
                ce = s["ce"]
                nc.vector.tensor_sub(ce, conf_p, iou)
                nc.vector.tensor_mul(ce, ce, msk)
                nc.scalar.activation(ce, ce, AF.Square, scale=1.0,
                                     accum_out=slot(q, 1))

            def coord_phase(nc, q, sub_eng, mul_eng):
                g = GRPS[q]
                s = st[q]
                pv8 = pb[q][:, :, :, :].rearrange(
                    "p g (h c) s -> p g h c s", h=2)[:, :, :, 0:4, :]
                tv8 = tb[q][:, :, :, :].rearrange(
                    "p g (h c) s -> p g h c s", h=2)[:, :, :, 0:4, :]
                e8 = T(q, "e8", [P, g, 2, 4, SS])
                sub_eng.tensor_tensor(e8, pv8, tv8, OP.subtract)
                mskb = s["msk"][:, :, :, :].unsqueeze(3).broadcast_to(
                    [P, g, 2, 4, SS])
                mul_eng.tensor_tensor(e8, e8, mskb, OP.mult)
                nc.scalar.activation(e8, e8, AF.Square, scale=SQ5,
                                     accum_out=slot(q, 2))

            def class_chunk(nc, q, j, sub_eng, mul_eng):
                g = GRPS[q]
                w = CLS_CHUNKS[q][j]
                t4 = tb[q][:, :, :, :][:, :, 4, :]
                e = tcl[q][j]
                sub_eng.tensor_tensor(e, pc[q][j], e, OP.subtract)
                t4b = t4.unsqueeze(2).broadcast_to([P, g, w, SS])
                mul_eng.tensor_tensor(e, e, t4b, OP.mult)
                nc.scalar.activation(e, e, AF.Square, scale=1.0,
                                     accum_out=slot(q, 3 + j))

            V, G = nc.vector, nc.gpsimd

            # ---- emission order (per-engine queues are in-order),
            # ---- interleaved by expected readiness ----
            box_phase(nc, 0)
            box_phase(nc, 1)
            coord_phase(nc, 0, G, V)
            box_phase(nc, 2)
            class_chunk(nc, 0, 0, G, V)
            class_chunk(nc, 0, 1, G, V)
            coord_phase(nc, 1, G, V)
            class_chunk(nc, 0, 2, G, V)
            class_chunk(nc, 0, 3, G, V)
            coord_phase(nc, 2, G, V)
            class_chunk(nc, 0, 4, G, V)
            for j in range(len(CLS_CHUNKS[1])):
                sub_eng, mul_eng = (G, G) if j == 2 else (G, V)
                class_chunk(nc, 1, j, sub_eng, mul_eng)
            for j in range(len(CLS_CHUNKS[2])):
                # short tail: final two chunks on disjoint engine chains
                eng = [(G, V), (G, V), (G, V), (G, V), (V, V), (V, V),
                       (G, G)][j]
                class_chunk(nc, 2, j, *eng)

            nc.scalar.dma_start(out=out[:, :], in_=acc)
    nc.compile()
    return nc


def _get_nc():
    if "nc" not in _CACHE:
        _CACHE["nc"] = _build()
    return _CACHE["nc"]


def kernel(pred: np.ndarray, target: np.ndarray) -> np.ndarray:
    nc = _get_nc()
    in_maps = []
    for k in range(NCORE):
        sl = slice(k * NPC, (k + 1) * NPC)
        in_maps.append({
            "pred": np.ascontiguousarray(pred[sl]).reshape(NPC, D * SS),
            "target": np.ascontiguousarray(target[sl]).reshape(NPC, D * SS),
        })
    res = run_bass_kernel_spmd(nc, in_maps, core_ids=list(range(NCORE)))
    total = sum(float(r["out"].astype(np.float64).sum()) for r in res.results)
    return np.float32(total / N)
